# revision 52
# baseline (speedup 1.0000x reference)
"""Trainium2 Bass kernel: multi-head attention (B=4, T=2048, D=768, H=12).

Sharding: 8 cores = 4 batches x 2 head-groups (6 heads each).
Each core computes QKV projection (its heads), attention, and per-PAIR
partial output projections (contraction over each pair's 128 w_out
rows). Host unshard: out[b] = sum over 3 pairs of partial[2b] + same for
partial[2b+1] + b_out.

v3 design (290us baseline). The wall is jointly set by the PE and the
Scalar engine's exp throughput (192 exp tiles of [128,1024] at
(N+352)/1.2ns = 220us total, irreducible). PE work is cut below that
line by row-tiling the S matmuls: S^T contraction is head_dim=64, so
the S matmuls for kb-pair (2m, 2m+1) of the same head run as two
concurrent 64-contraction tiles on array rows 0:64 / 64:128
(tile_position auto-derived from operand base partitions). This needs
the K^T/Q^T pair tiles in both natural and partition-swapped layouts
(QTs/KTs built by SBUF->SBUF DMA after the projection evictions).
Measured on HW: 760ns vs 860ns per kb including the ~97ns tiled<->full
transition penalty (so all tiled work is grouped per kb-pair slot: the
4 S matmuls back-to-back, then full-array PV/fills).

Normalization is the measured-cheapest path (a tensor_scalar PSUM
eviction alternative cost +190ns/exp of ACT PSUM-read contention and
serialized the PE behind DVE): denominators from the V~ ones column
land in au row 64, reshaped [1,1024]->[8,128] by DMA so the reciprocal
runs on 8 DVE lanes, DMA'd back to a row, broadcast to 64 partitions
with contraction-1 matmuls and multiplied into AN (bf16) one unit
deferred, so the PE never waits on the reciprocal chain.

Out-projection per pair: a single 128-contraction matmul per token-tile
half against WO[p], plain-CAST eviction, DMA'd as a per-pair partial
(summed with the batch's other core partials on host). The tail after
the last exp is only the last pair's normalize chain + its 8 tiles.

Startup: input DMA priority order + a long dummy-matmul stream keeps
the PE HAM-warm through the DMA wait (the baseline idled 4.6us and
paid a re-throttle, running the projection preamble at 1.2GHz).

This walrus build encodes at most one sync wait per instruction; Tile
emits several. _split_multi_waits() rewrites the final module, hoisting
extra waits onto same-engine nops inserted before the instruction.
"""

import numpy as np

import concourse.bass as bass
import concourse.mybir as mybir
from concourse.tile import TileContext
from concourse.bass_utils import run_bass_kernel_spmd

# problem constants (fixed by the graded nn.Module)
B, T, D = 4, 2048, 768
H, HD = 12, 64
NCORES = 8
HL = H // 2            # heads per core (2 head-groups)
NPAIR = HL // 2        # head pairs per core

F32 = mybir.dt.float32
F32R = mybir.dt.float32r
BF16 = mybir.dt.bfloat16


def _split_multi_waits(nc):
    """Walrus here encodes only one sync wait per instruction. Move extra
    waits onto same-engine nops placed immediately before the instruction."""
    n = 0
    for f in nc.m.functions:
        for bb in f.blocks:
            new = []
            for inst in bb.instructions:
                si = inst.sync_info
                if si is not None and si.on_wait and len(si.on_wait) > 1:
                    extra = list(si.on_wait[:-1])
                    keep = si.on_wait[-1]
                    del si.on_wait[:]
                    si.on_wait.append(keep)
                    for w in extra:
                        nop = mybir.InstNoOp(name=f"I-wsplit-{n}", ins=[], outs=[])
                        n += 1
                        nop.engine = inst.engine
                        nop.sync_info = mybir.SyncInfo(on_wait=[w], on_update=[])
                        new.append(nop)
                new.append(inst)
            bb.instructions[:] = new
    return n


def build_nc(t=T, qc=1024, nch=512):
    """Build the SPMD per-core program. qc = attention query chunk,
    nch = matmul moving-dim chunk."""
    tokt = t // 128            # token tiles
    nqc = t // qc              # query chunks
    dk = D // 128              # contraction tiles over D
    ncc = t // nch             # projection moving chunks per M row
    nmt = 2 * HL * HD // 128   # QK projection M-tiles (6)
    qtt = qc // 128            # token tiles per query chunk (8)

    nc = bass.Bass("TRN2", target_bir_lowering=False, debug=False)

    xt_d = nc.dram_tensor("xt", [D, t], BF16, kind="ExternalInput")
    wqk_d = nc.dram_tensor("wqk", [D, 2 * HL * HD], BF16, kind="ExternalInput")
    bqk_d = nc.dram_tensor("bqk", [128, nmt], F32, kind="ExternalInput")
    wv_d = nc.dram_tensor("wv", [D + 1, HL * HD], BF16, kind="ExternalInput")
    wo_d = nc.dram_tensor("wo", [HL * HD, D], BF16, kind="ExternalInput")
    # per-pair output partials: pair p occupies rows [p*t, (p+1)*t)
    out_d = nc.dram_tensor("out", [NPAIR * t, D], BF16, kind="ExternalOutput")

    def MM(out, lhsT, rhs, start, stop):
        nc.tensor.matmul(out, lhsT, rhs, start=start, stop=stop)

    with TileContext(nc) as tc:
        lp = nc.allow_low_precision(reason="bf16/f32r matmul operand production")
        lp.__enter__()
        with tc.tile_pool(name="persist", bufs=1) as pp:
            ones_row = pp.tile([1, 128], F32R, name="ones_row")
            ones_bf = pp.tile([1, 128], BF16, name="ones_bf")
            warm_sb = pp.tile([128, 512], BF16, name="warm_sb")
            QT = [pp.tile([128, t], BF16, name=f"qt{p}") for p in range(NPAIR)]
            QTs = [pp.tile([128, t], BF16, name=f"qts{p}") for p in range(NPAIR)]
            KT = [pp.tile([128, t], BF16, name=f"kt{p}") for p in range(NPAIR)]
            KTs = [pp.tile([128, t], BF16, name=f"kts{p}") for p in range(NPAIR)]
            V6 = [pp.tile([128, HL * (HD + 1)], BF16, name=f"v6_{c}") for c in range(tokt)]
            bqk_t = pp.tile([128, nmt], F32, name="bqk_t")
            AN = [pp.tile([128, t], BF16, name=f"an{p}") for p in range(NPAIR)]
            WO = [pp.tile([128, D], BF16, name=f"wop{p}") for p in range(NPAIR)]
            r_pads = [pp.tile([1, qc], F32R, name=f"r_pad{i}") for i in range(2)]
            au_sbs = [pp.tile([65, qc], F32, name=f"au_sb{i}") for i in range(2)]
            den8s = [pp.tile([8, qc // 8], F32, name=f"den8_{i}") for i in range(2)]
            rec8s = [pp.tile([8, qc // 8], F32R, name=f"rec8_{i}") for i in range(2)]
            xt_t = pp.tile([128, dk, t], BF16, name="xt_t")
            wqk_t = pp.tile([128, dk, 2 * HL * HD], BF16, name="wqk_t")
            wv_t = pp.tile([128, dk, HL * HD], BF16, name="wv_t")
            wvb = pp.tile([1, HL * HD], BF16, name="wvb")
            wvb_full = pp.tile([128, HL * HD], BF16, name="wvb_full")

            # ---- DMA emission in priority order: the first S matmul needs
            # KT[0] chunk0 + QT[0] cols 0:1024 (xt chunks 0-1 + wqk pair0);
            # V~ tiles need wv; xt chunks 2-3 aren't consumed until kb 8+.
            nc.sync.dma_start(out=bqk_t[:], in_=bqk_d[:, :])

            def dma_wqk(psl):
                nc.sync.dma_start(
                    out=wqk_t[:, :, psl],
                    in_=wqk_d[:, psl].rearrange("(k r) c -> r k c", k=dk),
                )

            def dma_xt(ch):
                csl = slice(ch * nch, (ch + 1) * nch)
                nc.sync.dma_start(
                    out=xt_t[:, :, csl],
                    in_=xt_d[:, csl].rearrange("(k r) c -> r k c", k=dk),
                )

            # critical first batch only: the rest is emitted AFTER the
            # projection preamble so the preamble's swap-DMA triggers are not
            # stuck behind ~18us of bulk input triggers on the sync queue.
            dma_wqk(slice(128, 256))
            dma_xt(0)
            dma_wqk(slice(0, 128))
            dma_xt(1)
            nc.sync.dma_start(
                out=wv_t[:], in_=wv_d[0:D, :].rearrange("(k r) c -> r k c", k=dk)
            )
            nc.sync.dma_start(out=wvb[0:1, :], in_=wv_d[D : D + 1, :])

            dma_xt(2)
            dma_xt(3)
            dma_wqk(slice(256, 512))
            dma_wqk(slice(512, 768))
            for p_ in range(NPAIR):
                nc.sync.dma_start(out=WO[p_][:], in_=wo_d[p_ * 128 : (p_ + 1) * 128, :])

            # ---- constants init ----
            with tc.tile_pool(name="init", bufs=1) as ip:
                ones32 = ip.tile([1, 128], F32, name="ones32")
                nc.vector.memset(ones32[:], 1.0)
                nc.vector.tensor_copy(ones_row[:], ones32[:])
                nc.vector.memset(ones_bf[:], 1.0)
                nc.vector.memset(warm_sb[:], 0.0)
                warm = ip.tile([1, 16], F32, name="warm")
                nc.scalar.activation(
                    warm[:], ones32[0:1, 0:16], mybir.ActivationFunctionType.Exp
                )
                # V~ ones columns: tiny strided memsets (6 els/lane) on DVE,
                # keeping the GpSimd queue free for the swap DMA triggers
                for c in range(tokt):
                    v3i = V6[c][:].rearrange("p (h c) -> p h c", c=HD + 1)
                    nc.vector.memset(v3i[:, :, HD : HD + 1], 1.0)

            with (
                tc.tile_pool(name="ps_s", bufs=2, space="PSUM") as s_pool,
                tc.tile_pool(name="ps_u", bufs=1, space="PSUM") as u_pool,
                tc.tile_pool(name="ps_x", bufs=2, space="PSUM") as x_pool,
                tc.tile_pool(name="sb_pt", bufs=8) as ptp,
                tc.tile_pool(name="sb_r", bufs=2) as rsp,
                tc.tile_pool(name="sb_o", bufs=3) as osp,
            ):
                # ---------- micro-item emitters ----------
                aux_state = {}

                def swap_dma(dst, src, csl):
                    # swap triggers ride the otherwise-idle GpSimd queue so
                    # they don't queue behind the bulk input DMA triggers
                    nc.gpsimd.dma_start(out=dst[0:64, csl], in_=src[64:128, csl])
                    nc.gpsimd.dma_start(out=dst[64:128, csl], in_=src[0:64, csl])

                def qk_half(p_, m, c, half):
                    """Half of one QK-projection chunk: 3 of 6 k-matmuls into
                    an aux PSUM slot; second half evicts to QT/KT + swap DMA."""
                    key = ("qk", p_, m, c)
                    gm = 2 * p_ + m
                    csl = slice(c * nch, (c + 1) * nch)
                    if half == 0:
                        ps = x_pool.tile([128, nch], F32, tag="x", bufs=2, name="psqk")
                        aux_state[key] = ps
                        ks = range(0, dk // 2)
                    else:
                        ps = aux_state.pop(key)
                        ks = range(dk // 2, dk)
                    for k in ks:
                        MM(
                            ps[:],
                            wqk_t[:, k, gm * 128 : (gm + 1) * 128],
                            xt_t[:, k, csl],
                            start=(k == 0),
                            stop=(k == dk - 1),
                        )
                    if half == 1:
                        dst = QT[p_] if m == 0 else KT[p_]
                        dsts = QTs[p_] if m == 0 else KTs[p_]
                        nc.vector.tensor_scalar_add(
                            dst[:, csl], ps[:], bqk_t[:, gm : gm + 1]
                        )
                        swap_dma(dsts, dst, csl)

                def v6_half(c, half):
                    """Half of one V~ tile build: k-matmuls into aux PSUM;
                    second half adds bias (contraction-1 matmul) and scatters
                    into V6[c] with the per-head ones column."""
                    key = ("v6", c)
                    tsl = slice(c * 128, (c + 1) * 128)
                    if half == 0:
                        psv = x_pool.tile(
                            [128, HL * HD], F32, tag="x", bufs=2, name="psv"
                        )
                        aux_state[key] = psv
                        for k in range(0, dk // 2):
                            MM(psv[:], xt_t[:, k, tsl], wv_t[:, k, :], start=(k == 0), stop=False)
                    else:
                        psv = aux_state.pop(key)
                        for k in range(dk // 2, dk):
                            MM(psv[:], xt_t[:, k, tsl], wv_t[:, k, :], start=False,
                               stop=(k == dk - 1))
                        # bias folded into the eviction (wvb pre-broadcast to
                        # 128 partitions once) - saves a contraction-1 matmul
                        # and its two tile-mode transitions per V~ tile
                        v3 = V6[c][:].rearrange("p (h c) -> p h c", c=HD + 1)
                        nc.vector.tensor_add(
                            v3[:, :, 0:HD],
                            psv[:].rearrange("p (h c) -> p h c", c=HD),
                            wvb_full[:].rearrange("p (h c) -> p h c", c=HD),
                        )

                def oproj_half(p_, q, c, hf, evict=None, dma_eng=None):
                    """Out-proj of one token tile half for pair p_: a single
                    128-contraction matmul against WO[p_] (AN pre-normalized),
                    CAST eviction into a staging tile, DMA per-pair partial.
                    evict engine is DVE by default; the tail alternates with
                    the then-idle ScalarE."""
                    key = ("op", p_, q, c)
                    t0 = q * qc + c * 128
                    tsl = slice(t0, t0 + 128)
                    nsl = slice(hf * (D // 2), (hf + 1) * (D // 2))
                    ps = x_pool.tile([128, D // 2], F32, tag="x", bufs=2, name="pso")
                    if hf == 0:
                        so = osp.tile([128, D], BF16, tag="so", bufs=3, name="so")
                        aux_state[key] = so
                    else:
                        so = aux_state.pop(key)
                    MM(ps[:], AN[p_][:, tsl], WO[p_][:, nsl], start=True, stop=True)
                    if evict is None:
                        nc.vector.tensor_copy(so[:, nsl], ps[:])
                    else:
                        evict(so[:, nsl], ps[:])
                    if hf == 1:
                        (dma_eng or nc.sync).dma_start(
                            out=out_d[p_ * t + t0 : p_ * t + t0 + 128, :], in_=so[:]
                        )

                def finish_unit(u):
                    """Deferred normalize: broadcast the reciprocal row to 64
                    partitions (on the otherwise-idle GpSimd engine, keeping
                    the PE out of the chain), multiply into AN."""
                    up, uj, uq, uau_sb, urp = u
                    uqsl = slice(uq * qc, (uq + 1) * qc)
                    R_sb = rsp.tile([64, qc], F32, tag="rsb", bufs=2, name="R_sb")
                    for c in range(qc // nch):
                        csl = slice(c * nch, (c + 1) * nch)
                        R = x_pool.tile([64, nch], F32, tag="x", bufs=2, name="Rp")
                        MM(R[:], ones_row[0:1, 0:64], urp[0:1, csl], start=True, stop=True)
                        nc.vector.tensor_copy(R_sb[:, csl], R[:])
                    nc.vector.tensor_mul(
                        AN[up][uj * 64 : (uj + 1) * 64, uqsl], uau_sb[0:64, :], R_sb[:]
                    )

                # ---------- fill schedules ----------
                def v6_items():
                    return [
                        (lambda c=c, hf=hf: v6_half(c, hf))
                        for c in range(tokt)
                        for hf in range(2)
                    ]

                def qk_items(p_, m, cs):
                    return [
                        (lambda c=c, hf=hf: qk_half(p_, m, c, hf))
                        for c in cs
                        for hf in range(2)
                    ]

                def op_items(p_, q):
                    return [
                        (lambda c=c, hf=hf: oproj_half(p_, q, c, hf))
                        for c in range(qtt)
                        for hf in range(2)
                    ]

                v6h = v6_items()
                fills = {i: [] for i in range(2 * HL)}
                # unit 0 absorbs the rest of KT pair0 (chunk c ready before
                # the kbs that consume it) and all V~ builds (V6[c] ready
                # before the trailing PV(c))
                fills[0] = (
                    qk_items(0, 1, [1]) + v6h[0:4]
                    + qk_items(0, 1, [2]) + v6h[4:12]
                    + qk_items(0, 1, [3]) + v6h[12:32]
                )
                # oproj(p, q) reads AN[p] fully normalized, which happens at
                # pair m==5 of unit 2k+2 (k = q*NPAIR+p): schedule its items
                # from unit 2k+3 on.
                op00 = op_items(0, 0)
                op10 = op_items(1, 0)
                op20 = op_items(2, 0)
                op01 = op_items(0, 1)
                fills[1] = qk_items(1, 1, [0, 1]) + qk_items(1, 0, [0, 1])
                fills[2] = qk_items(1, 1, [2, 3]) + qk_items(2, 1, [0, 1])
                fills[3] = qk_items(2, 1, [2, 3]) + qk_items(2, 0, [0, 1]) + op00[:2]
                fills[4] = qk_items(1, 0, [2, 3]) + op00[2:8]
                fills[5] = qk_items(0, 0, [2, 3]) + op00[8:] + op10[:4]
                fills[6] = qk_items(2, 0, [2, 3]) + op10[4:12]
                fills[7] = op10[12:] + op20[:10]
                fills[8] = op20[10:]
                fills[9] = op01[:12]
                fills[10] = op01[12:]
                fills[11] = op_items(1, 1)

                # ---- PE p-state warmup: a long dummy-matmul stream keeps
                # the HAM warm through the input-DMA wait so the projection
                # preamble and first S run at full clock.
                wps = x_pool.tile([128, 128], F32, tag="x", bufs=2, name="wps")
                for wi in range(56):
                    MM(
                        wps[:],
                        warm_sb[:, 0:128],
                        warm_sb[:, 0:128],
                        start=(wi == 0),
                        stop=(wi == 55),
                    )

                # ---- projection preamble: KT pair0 chunk0, QT pair0 q0 ----
                qk_half(0, 1, 0, 0)
                qk_half(0, 1, 0, 1)
                for c in (0, 1):
                    qk_half(0, 0, c, 0)
                    qk_half(0, 0, c, 1)
                # one-time broadcast of the V bias row to all 128 partitions
                psb0 = x_pool.tile([128, HL * HD], F32, tag="x", bufs=2, name="psb0")
                MM(psb0[:], ones_bf[0:1, 0:128], wvb[0:1, :], start=True, stop=True)
                nc.vector.tensor_copy(wvb_full[:], psb0[:])

                # ---- attention units ----
                units = [
                    (q, p_, j)
                    for q in range(nqc)
                    for p_ in range(NPAIR)
                    for j in range(2)
                ]
                pending = None
                unit_no = 0
                for ui, (q, p_, j) in enumerate(units):
                    fl = fills[ui]
                    au = u_pool.tile([65, qc], F32, tag="au", bufs=1, name="au")
                    h = 2 * p_ + j
                    vsl = slice(h * (HD + 1), (h + 1) * (HD + 1))

                    def emit_pv(okb, pt_c0, pt_c1):
                        MM(
                            au[:, 0:nch], V6[okb][:, vsl], pt_c0[:, 0:nch],
                            start=(okb == 0), stop=(okb == tokt - 1),
                        )
                        MM(
                            au[:, nch:qc], V6[okb][:, vsl], pt_c1[:, nch:qc],
                            start=(okb == 0), stop=(okb == tokt - 1),
                        )

                    # tiled S sources: tile A (rows 0:64) needs head h's K/Q
                    # at partitions 0:64; tile B (rows 64:128) at 64:128.
                    if j == 0:
                        ktA, ktB = KT[p_], KTs[p_]
                        qtA, qtB = QT[p_], QTs[p_]
                    else:
                        ktA, ktB = KTs[p_], KT[p_]
                        qtA, qtB = QTs[p_], QT[p_]

                    # PV trails exp so the in-order PE queue has ready work
                    # while exps run; larger lag in unit 0 for V~ JIT fills.
                    L = 6 if ui == 0 else (2 if ui == 11 else 3)
                    pvq = []
                    npair_kb = tokt // 2
                    for m in range(npair_kb):
                        if m == 5 and pending is not None:
                            finish_unit(pending)
                            pending = None
                        a, b_ = 2 * m, 2 * m + 1
                        asl = slice(a * 128, (a + 1) * 128)
                        bsl = slice(b_ * 128, (b_ + 1) * 128)
                        qlo = slice(q * qc, q * qc + nch)
                        qhi = slice(q * qc + nch, q * qc + qc)
                        # co-located concurrent tiles: both MMs of a pair
                        # write the SAME st buffer (different banks), so both
                        # wait on the same prior exp and issue together.
                        # X = [A: kb a, q-lo | B: kb b, q-hi]
                        # Y = [A: kb b, q-lo | B: kb a, q-hi]
                        st_x = s_pool.tile([128, qc], F32, tag="st", bufs=2, name="stx")
                        st_y = s_pool.tile([128, qc], F32, tag="st", bufs=2, name="sty")
                        pt_x = ptp.tile([128, qc], BF16, tag="pt", bufs=8, name="ptx")
                        pt_y = ptp.tile([128, qc], BF16, tag="pt", bufs=8, name="pty")
                        if ui == 0 and m < 3:
                            # startup special: tile-A-only (natural layouts),
                            # so the first exps don't wait for the swap DMAs
                            MM(st_x[:, 0:nch], ktA[0:64, asl], qtA[0:64, qlo],
                               start=True, stop=True)
                            MM(st_x[:, nch:qc], ktA[0:64, asl], qtA[0:64, qhi],
                               start=True, stop=True)
                            nc.scalar.activation(
                                pt_x[:], st_x[:], mybir.ActivationFunctionType.Exp, scale=0.125
                            )
                            MM(st_y[:, 0:nch], ktA[0:64, bsl], qtA[0:64, qlo],
                               start=True, stop=True)
                            MM(st_y[:, nch:qc], ktA[0:64, bsl], qtA[0:64, qhi],
                               start=True, stop=True)
                            nc.scalar.activation(
                                pt_y[:], st_y[:], mybir.ActivationFunctionType.Exp, scale=0.125
                            )
                            pvq.append((a, pt_x, pt_x))
                            pvq.append((b_, pt_y, pt_y))
                        else:
                            MM(st_x[:, 0:nch], ktA[0:64, asl], qtA[0:64, qlo],
                               start=True, stop=True)
                            MM(st_x[:, nch:qc], ktB[64:128, bsl], qtB[64:128, qhi],
                               start=True, stop=True)
                            nc.scalar.activation(
                                pt_x[:], st_x[:], mybir.ActivationFunctionType.Exp, scale=0.125
                            )
                            MM(st_y[:, 0:nch], ktA[0:64, bsl], qtA[0:64, qlo],
                               start=True, stop=True)
                            MM(st_y[:, nch:qc], ktB[64:128, asl], qtB[64:128, qhi],
                               start=True, stop=True)
                            nc.scalar.activation(
                                pt_y[:], st_y[:], mybir.ActivationFunctionType.Exp, scale=0.125
                            )
                            pvq.append((a, pt_x, pt_y))
                            pvq.append((b_, pt_y, pt_x))
                        # evenly drain this unit's fills across its 8 pairs
                        left = npair_kb - m
                        npop = (len(fl) + left - 1) // left if fl else 0
                        if ui == 0:
                            npop = min(npop, 5)
                        for _ in range(npop):
                            if fl:
                                fl.pop(0)()
                        lag = L if (ui == 0 or m < 6) else 1
                        while len(pvq) > lag:
                            emit_pv(*pvq.pop(0))
                    while fl:
                        fl.pop(0)()
                    for ent in pvq:
                        emit_pv(*ent)
                    # unit end: evict au, launch the reciprocal chain (the
                    # last unit defers to the pipelined tail version below)
                    if ui < 2 * HL - 1:
                        au_sb = au_sbs[unit_no % 2]
                        nc.vector.tensor_copy(au_sb[:], au[:])
                        rp_t = r_pads[unit_no % 2]
                        den8 = den8s[unit_no % 2]
                        rec8 = rec8s[unit_no % 2]
                        nc.sync.dma_start(out=den8[:], in_=au_sb[64:65, :])
                        nc.vector.reciprocal(rec8[:], den8[:])
                        nc.sync.dma_start(out=rp_t[0:1, :], in_=rec8[:])
                        if pending is not None:
                            finish_unit(pending)
                        pending = (p_, j, q, au_sb, rp_t)
                    else:
                        last_au = au
                    unit_no += 1
                if pending is not None:
                    finish_unit(pending)

                # ---- pipelined normalize for the last unit (p2, j1, q1):
                # per q-half so the tail out-projection of tiles 0-3 starts
                # while the second half's reciprocal chain is still running
                au_sb = au_sbs[1]
                rp_t = r_pads[1]
                R_sbt = rsp.tile([64, qc], F32, tag="rsb", bufs=2, name="R_sbt")
                for ch in range(2):
                    csl = slice(ch * nch, (ch + 1) * nch)
                    # per-half den/rec use the two parity tiles (rows 0:4) so
                    # every engine op keeps base partition 0
                    den8 = den8s[ch]
                    rec8 = rec8s[ch]
                    nc.vector.tensor_copy(au_sb[0:65, csl], last_au[:, csl])
                    nc.sync.dma_start(out=den8[0:4, :], in_=au_sb[64:65, csl])
                    nc.vector.reciprocal(rec8[0:4, :], den8[0:4, :])
                    nc.sync.dma_start(out=rp_t[0:1, csl], in_=rec8[0:4, :])
                    Rt = x_pool.tile([64, nch], F32, tag="x", bufs=2, name="Rt")
                    MM(Rt[:], ones_row[0:1, 0:64], rp_t[0:1, csl], start=True, stop=True)
                    nc.vector.tensor_copy(R_sbt[:, csl], Rt[:])
                    nc.vector.tensor_mul(
                        AN[2][64:128, qc + ch * nch : qc + (ch + 1) * nch],
                        au_sb[0:64, csl], R_sbt[:, csl],
                    )

                # ---- tail: the last pair's out-projection (q1); evictions
                # alternate DVE / ScalarE (idle after the last exp), and the
                # final out DMAs alternate sync/gpsimd queues to drain 2x
                for c in range(qtt):
                    for hf in range(2):
                        ev = nc.scalar.copy if (c + hf) % 2 else None
                        oproj_half(2, 1, c, hf, evict=ev,
                                   dma_eng=(nc.gpsimd if c % 2 else nc.sync))
        lp.__exit__(None, None, None)

    return nc


def shard_inputs(x, w_qkv, b_qkv, w_out, b_out, t=T):
    """Build the 8 per-core input maps. Core = (batch, head-group)."""
    in_maps = []
    for core in range(NCORES):
        b, g = divmod(core, 2)
        hbase = HL * g * HD          # first qk column of this group (384*g)
        # q cols then k cols, pair-interleaved: M-tile 2p = q of heads (2p,2p+1),
        # M-tile 2p+1 = k of the same heads.
        wqk = np.empty((D, 2 * HL * HD), dtype=np.float32)
        bqk = np.empty((2 * HL * HD,), dtype=np.float32)
        for p in range(NPAIR):
            qcols = slice(0 * D + hbase + p * 128, 0 * D + hbase + (p + 1) * 128)
            kcols = slice(1 * D + hbase + p * 128, 1 * D + hbase + (p + 1) * 128)
            wqk[:, (2 * p) * 128 : (2 * p + 1) * 128] = w_qkv[:, qcols]
            wqk[:, (2 * p + 1) * 128 : (2 * p + 2) * 128] = w_qkv[:, kcols]
            bqk[(2 * p) * 128 : (2 * p + 1) * 128] = b_qkv[qcols]
            bqk[(2 * p + 1) * 128 : (2 * p + 2) * 128] = b_qkv[kcols]
        nmt = 2 * HL * HD // 128
        bqk_col = np.ascontiguousarray(bqk.reshape(nmt, 128).T)  # [128, nmt]

        vcols = slice(2 * D + hbase, 2 * D + hbase + HL * HD)
        wv = np.empty((D + 1, HL * HD), dtype=np.float32)
        wv[:D] = w_qkv[:, vcols]
        wv[D] = b_qkv[vcols]

        import ml_dtypes

        bf = ml_dtypes.bfloat16
        wo = np.ascontiguousarray(w_out[hbase : hbase + HL * HD, :]).astype(bf)

        in_maps.append(
            {
                "xt": np.ascontiguousarray(x[b, :t].T).astype(bf),
                "wqk": wqk.astype(bf),
                "bqk": bqk_col,
                "wv": wv.astype(bf),
                "wo": wo,
            }
        )
    return in_maps


def unshard(parts, b_out):
    """parts[core] is [NPAIR*t, D] of per-pair partials; sum pairs, then
    the two cores of each batch, then add the output bias."""
    out = np.empty((B, T, D), dtype=np.float32)
    for b in range(B):
        lo = parts[2 * b].reshape(NPAIR, T, D).sum(axis=0)
        hi = parts[2 * b + 1].reshape(NPAIR, T, D).sum(axis=0)
        out[b] = lo + hi
    out += b_out[None, None, :]
    return out


def kernel(x, w_qkv, b_qkv, w_out, b_out):
    x = np.asarray(x, dtype=np.float32)
    w_qkv = np.asarray(w_qkv, dtype=np.float32)
    b_qkv = np.asarray(b_qkv, dtype=np.float32)
    w_out = np.asarray(w_out, dtype=np.float32)
    b_out = np.asarray(b_out, dtype=np.float32)

    nc = build_nc()
    _split_multi_waits(nc)
    in_maps = shard_inputs(x, w_qkv, b_qkv, w_out, b_out)
    res = run_bass_kernel_spmd(nc, in_maps, list(range(NCORES)))
    parts = [np.asarray(res.results[i]["out"]).astype(np.float32) for i in range(NCORES)]
    return unshard(parts, b_out).astype(np.float32)


# revision 53
# speedup vs baseline: 1.0196x; 1.0196x over previous
"""Trainium2 Bass kernel: multi-head attention (B=4, T=2048, D=768, H=12).

Sharding: 8 cores = 4 batches x 2 head-groups (6 heads each).
Each core computes QKV projection (its heads), attention, and per-PAIR
partial output projections (contraction over each pair's 128 w_out
rows). Host unshard: out[b] = sum over 3 pairs of partial[2b] + same for
partial[2b+1] + b_out.

v3 design (290us baseline). The wall is jointly set by the PE and the
Scalar engine's exp throughput (192 exp tiles of [128,1024] at
(N+352)/1.2ns = 220us total, irreducible). PE work is cut below that
line by row-tiling the S matmuls: S^T contraction is head_dim=64, so
the S matmuls for kb-pair (2m, 2m+1) of the same head run as two
concurrent 64-contraction tiles on array rows 0:64 / 64:128
(tile_position auto-derived from operand base partitions). This needs
the K^T/Q^T pair tiles in both natural and partition-swapped layouts
(QTs/KTs built by SBUF->SBUF DMA after the projection evictions).
Measured on HW: 760ns vs 860ns per kb including the ~97ns tiled<->full
transition penalty (so all tiled work is grouped per kb-pair slot: the
4 S matmuls back-to-back, then full-array PV/fills).

Normalization is the measured-cheapest path (a tensor_scalar PSUM
eviction alternative cost +190ns/exp of ACT PSUM-read contention and
serialized the PE behind DVE): denominators from the V~ ones column
land in au row 64, reshaped [1,1024]->[8,128] by DMA so the reciprocal
runs on 8 DVE lanes, DMA'd back to a row, broadcast to 64 partitions
with contraction-1 matmuls and multiplied into AN (bf16) one unit
deferred, so the PE never waits on the reciprocal chain.

Out-projection per pair: a single 128-contraction matmul per token-tile
half against WO[p], plain-CAST eviction, DMA'd as a per-pair partial
(summed with the batch's other core partials on host). The tail after
the last exp is only the last pair's normalize chain + its 8 tiles.

Startup: input DMA priority order + a long dummy-matmul stream keeps
the PE HAM-warm through the DMA wait (the baseline idled 4.6us and
paid a re-throttle, running the projection preamble at 1.2GHz).

This walrus build encodes at most one sync wait per instruction; Tile
emits several. _split_multi_waits() rewrites the final module, hoisting
extra waits onto same-engine nops inserted before the instruction.
"""

import numpy as np

import concourse.bass as bass
import concourse.mybir as mybir
from concourse.tile import TileContext
from concourse.bass_utils import run_bass_kernel_spmd

# problem constants (fixed by the graded nn.Module)
B, T, D = 4, 2048, 768
H, HD = 12, 64
NCORES = 8
HL = H // 2            # heads per core (2 head-groups)
NPAIR = HL // 2        # head pairs per core

F32 = mybir.dt.float32
F32R = mybir.dt.float32r
BF16 = mybir.dt.bfloat16


def _split_multi_waits(nc):
    """Walrus here encodes only one sync wait per instruction. Move extra
    waits onto same-engine nops placed immediately before the instruction."""
    n = 0
    for f in nc.m.functions:
        for bb in f.blocks:
            new = []
            for inst in bb.instructions:
                si = inst.sync_info
                if si is not None and si.on_wait and len(si.on_wait) > 1:
                    extra = list(si.on_wait[:-1])
                    keep = si.on_wait[-1]
                    del si.on_wait[:]
                    si.on_wait.append(keep)
                    for w in extra:
                        nop = mybir.InstNoOp(name=f"I-wsplit-{n}", ins=[], outs=[])
                        n += 1
                        nop.engine = inst.engine
                        nop.sync_info = mybir.SyncInfo(on_wait=[w], on_update=[])
                        new.append(nop)
                new.append(inst)
            bb.instructions[:] = new
    return n


def build_nc(t=T, qc=1024, nch=512):
    """Build the SPMD per-core program. qc = attention query chunk,
    nch = matmul moving-dim chunk."""
    tokt = t // 128            # token tiles
    nqc = t // qc              # query chunks
    dk = D // 128              # contraction tiles over D
    ncc = t // nch             # projection moving chunks per M row
    nmt = 2 * HL * HD // 128   # QK projection M-tiles (6)
    qtt = qc // 128            # token tiles per query chunk (8)

    nc = bass.Bass("TRN2", target_bir_lowering=False, debug=False)

    xt_d = nc.dram_tensor("xt", [D, t], BF16, kind="ExternalInput")
    wqk_d = nc.dram_tensor("wqk", [D, 2 * HL * HD], BF16, kind="ExternalInput")
    bqk_d = nc.dram_tensor("bqk", [128, nmt], F32, kind="ExternalInput")
    wv_d = nc.dram_tensor("wv", [D + 1, HL * HD], BF16, kind="ExternalInput")
    wo_d = nc.dram_tensor("wo", [HL * HD, D], BF16, kind="ExternalInput")
    # per-pair output partials: pair p occupies rows [p*t, (p+1)*t)
    out_d = nc.dram_tensor("out", [NPAIR * t, D], BF16, kind="ExternalOutput")

    def MM(out, lhsT, rhs, start, stop):
        nc.tensor.matmul(out, lhsT, rhs, start=start, stop=stop)

    with TileContext(nc) as tc:
        lp = nc.allow_low_precision(reason="bf16/f32r matmul operand production")
        lp.__enter__()
        with tc.tile_pool(name="persist", bufs=1) as pp:
            ones_row = pp.tile([1, 128], F32R, name="ones_row")
            ones_bf = pp.tile([1, 128], BF16, name="ones_bf")
            warm_sb = pp.tile([128, 512], BF16, name="warm_sb")
            QT = [pp.tile([128, t], BF16, name=f"qt{p}") for p in range(NPAIR)]
            QTs = [pp.tile([128, t], BF16, name=f"qts{p}") for p in range(NPAIR)]
            KT = [pp.tile([128, t], BF16, name=f"kt{p}") for p in range(NPAIR)]
            KTs = [pp.tile([128, t], BF16, name=f"kts{p}") for p in range(NPAIR)]
            V6 = [pp.tile([128, HL * (HD + 1)], BF16, name=f"v6_{c}") for c in range(tokt)]
            bqk_t = pp.tile([128, nmt], F32, name="bqk_t")
            AN = [pp.tile([128, t], BF16, name=f"an{p}") for p in range(NPAIR)]
            WO = [pp.tile([128, D], BF16, name=f"wop{p}") for p in range(NPAIR)]
            r_pads = [pp.tile([1, qc], F32R, name=f"r_pad{i}") for i in range(2)]
            au_sbs = [pp.tile([65, qc], F32, name=f"au_sb{i}") for i in range(2)]
            den8s = [pp.tile([8, qc // 8], F32, name=f"den8_{i}") for i in range(2)]
            rec8s = [pp.tile([8, qc // 8], F32R, name=f"rec8_{i}") for i in range(2)]
            xt_t = pp.tile([128, dk, t], BF16, name="xt_t")
            wqk_t = pp.tile([128, dk, 2 * HL * HD], BF16, name="wqk_t")
            wv_t = pp.tile([128, dk, HL * HD], BF16, name="wv_t")
            wvb = pp.tile([1, HL * HD], BF16, name="wvb")
            wvb_full = pp.tile([128, HL * HD], BF16, name="wvb_full")

            # ---- DMA emission in priority order: the first S matmul needs
            # KT[0] chunk0 + QT[0] cols 0:1024 (xt chunks 0-1 + wqk pair0);
            # V~ tiles need wv; xt chunks 2-3 aren't consumed until kb 8+.
            nc.sync.dma_start(out=bqk_t[:], in_=bqk_d[:, :])

            def dma_wqk(psl):
                nc.sync.dma_start(
                    out=wqk_t[:, :, psl],
                    in_=wqk_d[:, psl].rearrange("(k r) c -> r k c", k=dk),
                )

            def dma_xt(ch):
                csl = slice(ch * nch, (ch + 1) * nch)
                nc.sync.dma_start(
                    out=xt_t[:, :, csl],
                    in_=xt_d[:, csl].rearrange("(k r) c -> r k c", k=dk),
                )

            # critical first batch only: the rest is emitted AFTER the
            # projection preamble so the preamble's swap-DMA triggers are not
            # stuck behind ~18us of bulk input triggers on the sync queue.
            dma_wqk(slice(128, 256))
            dma_xt(0)
            dma_wqk(slice(0, 128))
            dma_xt(1)
            nc.sync.dma_start(
                out=wv_t[:], in_=wv_d[0:D, :].rearrange("(k r) c -> r k c", k=dk)
            )
            nc.sync.dma_start(out=wvb[0:1, :], in_=wv_d[D : D + 1, :])

            dma_xt(2)
            dma_xt(3)
            dma_wqk(slice(256, 512))
            dma_wqk(slice(512, 768))
            for p_ in range(NPAIR):
                nc.sync.dma_start(out=WO[p_][:], in_=wo_d[p_ * 128 : (p_ + 1) * 128, :])

            # ---- constants init ----
            with tc.tile_pool(name="init", bufs=1) as ip:
                ones32 = ip.tile([1, 128], F32, name="ones32")
                nc.vector.memset(ones32[:], 1.0)
                nc.vector.tensor_copy(ones_row[:], ones32[:])
                nc.vector.memset(ones_bf[:], 1.0)
                nc.vector.memset(warm_sb[:], 0.0)
                warm = ip.tile([1, 16], F32, name="warm")
                nc.scalar.activation(
                    warm[:], ones32[0:1, 0:16], mybir.ActivationFunctionType.Exp
                )
                # V~ ones columns: tiny strided memsets (6 els/lane) on DVE,
                # keeping the GpSimd queue free for the swap DMA triggers
                for c in range(tokt):
                    v3i = V6[c][:].rearrange("p (h c) -> p h c", c=HD + 1)
                    nc.vector.memset(v3i[:, :, HD : HD + 1], 1.0)

            with (
                tc.tile_pool(name="ps_s", bufs=2, space="PSUM") as s_pool,
                tc.tile_pool(name="ps_u", bufs=1, space="PSUM") as u_pool,
                tc.tile_pool(name="ps_x", bufs=2, space="PSUM") as x_pool,
                tc.tile_pool(name="sb_pt", bufs=8) as ptp,
                tc.tile_pool(name="sb_r", bufs=2) as rsp,
                tc.tile_pool(name="sb_o", bufs=3) as osp,
            ):
                # ---------- micro-item emitters ----------
                aux_state = {}

                def swap_dma(dst, src, csl):
                    # swap triggers ride the otherwise-idle GpSimd queue so
                    # they don't queue behind the bulk input DMA triggers
                    nc.gpsimd.dma_start(out=dst[0:64, csl], in_=src[64:128, csl])
                    nc.gpsimd.dma_start(out=dst[64:128, csl], in_=src[0:64, csl])

                def qk_half(p_, m, c, half):
                    """Half of one QK-projection chunk: 3 of 6 k-matmuls into
                    an aux PSUM slot; second half evicts to QT/KT + swap DMA."""
                    key = ("qk", p_, m, c)
                    gm = 2 * p_ + m
                    csl = slice(c * nch, (c + 1) * nch)
                    if half == 0:
                        ps = x_pool.tile([128, nch], F32, tag="x", bufs=2, name="psqk")
                        aux_state[key] = ps
                        ks = range(0, dk // 2)
                    else:
                        ps = aux_state.pop(key)
                        ks = range(dk // 2, dk)
                    for k in ks:
                        MM(
                            ps[:],
                            wqk_t[:, k, gm * 128 : (gm + 1) * 128],
                            xt_t[:, k, csl],
                            start=(k == 0),
                            stop=(k == dk - 1),
                        )
                    if half == 1:
                        dst = QT[p_] if m == 0 else KT[p_]
                        dsts = QTs[p_] if m == 0 else KTs[p_]
                        nc.vector.tensor_scalar_add(
                            dst[:, csl], ps[:], bqk_t[:, gm : gm + 1]
                        )
                        swap_dma(dsts, dst, csl)

                def v6_half(c, half):
                    """Half of one V~ tile build: k-matmuls into aux PSUM;
                    second half adds bias (contraction-1 matmul) and scatters
                    into V6[c] with the per-head ones column."""
                    key = ("v6", c)
                    tsl = slice(c * 128, (c + 1) * 128)
                    if half == 0:
                        psv = x_pool.tile(
                            [128, HL * HD], F32, tag="x", bufs=2, name="psv"
                        )
                        aux_state[key] = psv
                        for k in range(0, dk // 2):
                            MM(psv[:], xt_t[:, k, tsl], wv_t[:, k, :], start=(k == 0), stop=False)
                    else:
                        psv = aux_state.pop(key)
                        for k in range(dk // 2, dk):
                            MM(psv[:], xt_t[:, k, tsl], wv_t[:, k, :], start=False,
                               stop=(k == dk - 1))
                        # bias folded into the eviction (wvb pre-broadcast to
                        # 128 partitions once) - saves a contraction-1 matmul
                        # and its two tile-mode transitions per V~ tile
                        v3 = V6[c][:].rearrange("p (h c) -> p h c", c=HD + 1)
                        nc.vector.tensor_add(
                            v3[:, :, 0:HD],
                            psv[:].rearrange("p (h c) -> p h c", c=HD),
                            wvb_full[:].rearrange("p (h c) -> p h c", c=HD),
                        )

                def oproj_half(p_, q, c, hf, evict=None, dma_eng=None):
                    """Out-proj of one token tile half for pair p_: a single
                    128-contraction matmul against WO[p_] (AN pre-normalized),
                    CAST eviction into a staging tile, DMA per-pair partial.
                    evict engine is DVE by default; the tail alternates with
                    the then-idle ScalarE."""
                    key = ("op", p_, q, c)
                    t0 = q * qc + c * 128
                    tsl = slice(t0, t0 + 128)
                    nsl = slice(hf * (D // 2), (hf + 1) * (D // 2))
                    ps = x_pool.tile([128, D // 2], F32, tag="x", bufs=2, name="pso")
                    if hf == 0:
                        so = osp.tile([128, D], BF16, tag="so", bufs=3, name="so")
                        aux_state[key] = so
                    else:
                        so = aux_state.pop(key)
                    MM(ps[:], AN[p_][:, tsl], WO[p_][:, nsl], start=True, stop=True)
                    if evict is None:
                        nc.vector.tensor_copy(so[:, nsl], ps[:])
                    else:
                        evict(so[:, nsl], ps[:])
                    if hf == 1:
                        (dma_eng or nc.sync).dma_start(
                            out=out_d[p_ * t + t0 : p_ * t + t0 + 128, :], in_=so[:]
                        )

                def finish_unit(u):
                    """Deferred normalize: broadcast the reciprocal row to 64
                    partitions (on the otherwise-idle GpSimd engine, keeping
                    the PE out of the chain), multiply into AN."""
                    up, uj, uq, uau_sb, urp = u
                    uqsl = slice(uq * qc, (uq + 1) * qc)
                    R_sb = rsp.tile([64, qc], F32, tag="rsb", bufs=2, name="R_sb")
                    for c in range(qc // nch):
                        csl = slice(c * nch, (c + 1) * nch)
                        R = x_pool.tile([64, nch], F32, tag="x", bufs=2, name="Rp")
                        MM(R[:], ones_row[0:1, 0:64], urp[0:1, csl], start=True, stop=True)
                        nc.vector.tensor_copy(R_sb[:, csl], R[:])
                    nc.vector.tensor_mul(
                        AN[up][uj * 64 : (uj + 1) * 64, uqsl], uau_sb[0:64, :], R_sb[:]
                    )

                # ---------- fill schedules ----------
                def v6_items():
                    return [
                        (lambda c=c, hf=hf: v6_half(c, hf))
                        for c in range(tokt)
                        for hf in range(2)
                    ]

                def qk_items(p_, m, cs):
                    return [
                        (lambda c=c, hf=hf: qk_half(p_, m, c, hf))
                        for c in cs
                        for hf in range(2)
                    ]

                def op_items(p_, q):
                    return [
                        (lambda c=c, hf=hf: oproj_half(p_, q, c, hf))
                        for c in range(qtt)
                        for hf in range(2)
                    ]

                v6h = v6_items()
                fills = {i: [] for i in range(2 * HL)}
                # unit 0 absorbs the rest of KT pair0 (chunk c ready before
                # the kbs that consume it) and all V~ builds (V6[c] ready
                # before the trailing PV(c))
                fills[0] = (
                    qk_items(0, 1, [1]) + v6h[0:4]
                    + qk_items(0, 1, [2]) + v6h[4:12]
                    + qk_items(0, 1, [3]) + v6h[12:32]
                )
                # oproj(p, q) reads AN[p] fully normalized, which happens at
                # pair m==5 of unit 2k+2 (k = q*NPAIR+p): schedule its items
                # from unit 2k+3 on.
                op00 = op_items(0, 0)
                op10 = op_items(1, 0)
                op20 = op_items(2, 0)
                op01 = op_items(0, 1)
                fills[1] = qk_items(1, 1, [0, 1]) + qk_items(1, 0, [0, 1])
                fills[2] = qk_items(1, 1, [2, 3]) + qk_items(2, 1, [0, 1])
                fills[3] = qk_items(2, 1, [2, 3]) + qk_items(2, 0, [0, 1]) + op00[:2]
                fills[4] = qk_items(1, 0, [2, 3]) + op00[2:8]
                fills[5] = qk_items(0, 0, [2, 3]) + op00[8:] + op10[:4]
                fills[6] = qk_items(2, 0, [2, 3]) + op10[4:12]
                fills[7] = op10[12:] + op20[:10]
                fills[8] = op20[10:]
                fills[9] = op01[:12]
                fills[10] = op01[12:]
                fills[11] = op_items(1, 1)

                # ---- PE p-state warmup: a long dummy-matmul stream keeps
                # the HAM warm through the input-DMA wait so the projection
                # preamble and first S run at full clock.
                wps = x_pool.tile([128, 128], F32, tag="x", bufs=2, name="wps")
                for wi in range(56):
                    MM(
                        wps[:],
                        warm_sb[:, 0:128],
                        warm_sb[:, 0:128],
                        start=(wi == 0),
                        stop=(wi == 55),
                    )

                # ---- projection preamble: KT pair0 chunk0, QT pair0 q0 ----
                qk_half(0, 1, 0, 0)
                qk_half(0, 1, 0, 1)
                for c in (0, 1):
                    qk_half(0, 0, c, 0)
                    qk_half(0, 0, c, 1)
                # one-time broadcast of the V bias row to all 128 partitions
                psb0 = x_pool.tile([128, HL * HD], F32, tag="x", bufs=2, name="psb0")
                MM(psb0[:], ones_bf[0:1, 0:128], wvb[0:1, :], start=True, stop=True)
                nc.vector.tensor_copy(wvb_full[:], psb0[:])

                # ---- attention units ----
                units = [
                    (q, p_, j)
                    for q in range(nqc)
                    for p_ in range(NPAIR)
                    for j in range(2)
                ]
                pending = None
                unit_no = 0
                for ui, (q, p_, j) in enumerate(units):
                    fl = fills[ui]
                    au = u_pool.tile([65, qc], F32, tag="au", bufs=1, name="au")
                    h = 2 * p_ + j
                    vsl = slice(h * (HD + 1), (h + 1) * (HD + 1))

                    def emit_pv(okb, pt_c0, pt_c1):
                        MM(
                            au[:, 0:nch], V6[okb][:, vsl], pt_c0[:, 0:nch],
                            start=(okb == 0), stop=(okb == tokt - 1),
                        )
                        MM(
                            au[:, nch:qc], V6[okb][:, vsl], pt_c1[:, nch:qc],
                            start=(okb == 0), stop=(okb == tokt - 1),
                        )

                    # tiled S sources: tile A (rows 0:64) needs head h's K/Q
                    # at partitions 0:64; tile B (rows 64:128) at 64:128.
                    if j == 0:
                        ktA, ktB = KT[p_], KTs[p_]
                        qtA, qtB = QT[p_], QTs[p_]
                    else:
                        ktA, ktB = KTs[p_], KT[p_]
                        qtA, qtB = QTs[p_], QT[p_]

                    # PV trails exp so the in-order PE queue has ready work
                    # while exps run; larger lag in unit 0 for V~ JIT fills.
                    L = 6 if ui == 0 else (2 if ui == 11 else 3)
                    pvq = []
                    npair_kb = tokt // 2
                    for m in range(npair_kb):
                        if m == 5 and pending is not None:
                            finish_unit(pending)
                            pending = None
                        a, b_ = 2 * m, 2 * m + 1
                        asl = slice(a * 128, (a + 1) * 128)
                        bsl = slice(b_ * 128, (b_ + 1) * 128)
                        qlo = slice(q * qc, q * qc + nch)
                        qhi = slice(q * qc + nch, q * qc + qc)
                        # co-located concurrent tiles: both MMs of a pair
                        # write the SAME st buffer (different banks), so both
                        # wait on the same prior exp and issue together.
                        # X = [A: kb a, q-lo | B: kb b, q-hi]
                        # Y = [A: kb b, q-lo | B: kb a, q-hi]
                        st_x = s_pool.tile([128, qc], F32, tag="st", bufs=2, name="stx")
                        st_y = s_pool.tile([128, qc], F32, tag="st", bufs=2, name="sty")
                        pt_x = ptp.tile([128, qc], BF16, tag="pt", bufs=8, name="ptx")
                        pt_y = ptp.tile([128, qc], BF16, tag="pt", bufs=8, name="pty")
                        if ui == 0 and m < 3:
                            # startup special: tile-A-only (natural layouts),
                            # so the first exps don't wait for the swap DMAs
                            MM(st_x[:, 0:nch], ktA[0:64, asl], qtA[0:64, qlo],
                               start=True, stop=True)
                            MM(st_x[:, nch:qc], ktA[0:64, asl], qtA[0:64, qhi],
                               start=True, stop=True)
                            nc.scalar.activation(
                                pt_x[:], st_x[:], mybir.ActivationFunctionType.Exp, scale=0.125
                            )
                            MM(st_y[:, 0:nch], ktA[0:64, bsl], qtA[0:64, qlo],
                               start=True, stop=True)
                            MM(st_y[:, nch:qc], ktA[0:64, bsl], qtA[0:64, qhi],
                               start=True, stop=True)
                            nc.scalar.activation(
                                pt_y[:], st_y[:], mybir.ActivationFunctionType.Exp, scale=0.125
                            )
                            pvq.append((a, pt_x, pt_x))
                            pvq.append((b_, pt_y, pt_y))
                        else:
                            MM(st_x[:, 0:nch], ktA[0:64, asl], qtA[0:64, qlo],
                               start=True, stop=True)
                            MM(st_x[:, nch:qc], ktB[64:128, bsl], qtB[64:128, qhi],
                               start=True, stop=True)
                            nc.scalar.activation(
                                pt_x[:], st_x[:], mybir.ActivationFunctionType.Exp, scale=0.125
                            )
                            MM(st_y[:, 0:nch], ktA[0:64, bsl], qtA[0:64, qlo],
                               start=True, stop=True)
                            MM(st_y[:, nch:qc], ktB[64:128, asl], qtB[64:128, qhi],
                               start=True, stop=True)
                            nc.scalar.activation(
                                pt_y[:], st_y[:], mybir.ActivationFunctionType.Exp, scale=0.125
                            )
                            pvq.append((a, pt_x, pt_y))
                            pvq.append((b_, pt_y, pt_x))
                        # evenly drain this unit's fills across its 8 pairs
                        left = npair_kb - m
                        npop = (len(fl) + left - 1) // left if fl else 0
                        if ui == 0:
                            npop = min(npop, 5)
                        for _ in range(npop):
                            if fl:
                                fl.pop(0)()
                        lag = L if (ui == 0 or m < 6) else 1
                        while len(pvq) > lag:
                            emit_pv(*pvq.pop(0))
                    while fl:
                        fl.pop(0)()
                    for ent in pvq:
                        emit_pv(*ent)
                    # unit end: evict au, launch the reciprocal chain (the
                    # last unit defers to the pipelined tail version below)
                    if ui < 2 * HL - 1:
                        au_sb = au_sbs[unit_no % 2]
                        nc.vector.tensor_copy(au_sb[:], au[:])
                        rp_t = r_pads[unit_no % 2]
                        den8 = den8s[unit_no % 2]
                        rec8 = rec8s[unit_no % 2]
                        nc.sync.dma_start(out=den8[:], in_=au_sb[64:65, :])
                        nc.vector.reciprocal(rec8[:], den8[:])
                        nc.sync.dma_start(out=rp_t[0:1, :], in_=rec8[:])
                        if pending is not None:
                            finish_unit(pending)
                        pending = (p_, j, q, au_sb, rp_t)
                    else:
                        last_au = au
                    unit_no += 1
                if pending is not None:
                    finish_unit(pending)

                # ---- pipelined normalize for the last unit (p2, j1, q1):
                # per q-half so the tail out-projection of tiles 0-3 starts
                # while the second half's reciprocal chain is still running
                au_sb = au_sbs[1]
                rp_t = r_pads[1]
                R_sbt = rsp.tile([64, qc], F32, tag="rsb", bufs=2, name="R_sbt")
                for ch in range(2):
                    csl = slice(ch * nch, (ch + 1) * nch)
                    # per-half den/rec use the two parity tiles (rows 0:4) so
                    # every engine op keeps base partition 0
                    den8 = den8s[ch]
                    rec8 = rec8s[ch]
                    nc.vector.tensor_copy(au_sb[0:65, csl], last_au[:, csl])
                    nc.sync.dma_start(out=den8[0:4, :], in_=au_sb[64:65, csl])
                    nc.vector.reciprocal(rec8[0:4, :], den8[0:4, :])
                    nc.sync.dma_start(out=rp_t[0:1, csl], in_=rec8[0:4, :])
                    Rt = x_pool.tile([64, nch], F32, tag="x", bufs=2, name="Rt")
                    MM(Rt[:], ones_row[0:1, 0:64], rp_t[0:1, csl], start=True, stop=True)
                    nc.vector.tensor_copy(R_sbt[:, csl], Rt[:])
                    nc.vector.tensor_mul(
                        AN[2][64:128, qc + ch * nch : qc + (ch + 1) * nch],
                        au_sb[0:64, csl], R_sbt[:, csl],
                    )

                # ---- tail: the last pair's out-projection (q1); evictions
                # alternate DVE / ScalarE (idle after the last exp), and the
                # final out DMAs alternate sync/gpsimd queues to drain 2x
                for c in range(qtt):
                    for hf in range(2):
                        ev = nc.scalar.copy if (c + hf) % 2 else None
                        oproj_half(2, 1, c, hf, evict=ev)
        lp.__exit__(None, None, None)

    return nc


def shard_inputs(x, w_qkv, b_qkv, w_out, b_out, t=T):
    """Build the 8 per-core input maps. Core = (batch, head-group)."""
    in_maps = []
    for core in range(NCORES):
        b, g = divmod(core, 2)
        hbase = HL * g * HD          # first qk column of this group (384*g)
        # q cols then k cols, pair-interleaved: M-tile 2p = q of heads (2p,2p+1),
        # M-tile 2p+1 = k of the same heads.
        wqk = np.empty((D, 2 * HL * HD), dtype=np.float32)
        bqk = np.empty((2 * HL * HD,), dtype=np.float32)
        for p in range(NPAIR):
            qcols = slice(0 * D + hbase + p * 128, 0 * D + hbase + (p + 1) * 128)
            kcols = slice(1 * D + hbase + p * 128, 1 * D + hbase + (p + 1) * 128)
            wqk[:, (2 * p) * 128 : (2 * p + 1) * 128] = w_qkv[:, qcols]
            wqk[:, (2 * p + 1) * 128 : (2 * p + 2) * 128] = w_qkv[:, kcols]
            bqk[(2 * p) * 128 : (2 * p + 1) * 128] = b_qkv[qcols]
            bqk[(2 * p + 1) * 128 : (2 * p + 2) * 128] = b_qkv[kcols]
        nmt = 2 * HL * HD // 128
        bqk_col = np.ascontiguousarray(bqk.reshape(nmt, 128).T)  # [128, nmt]

        vcols = slice(2 * D + hbase, 2 * D + hbase + HL * HD)
        wv = np.empty((D + 1, HL * HD), dtype=np.float32)
        wv[:D] = w_qkv[:, vcols]
        wv[D] = b_qkv[vcols]

        import ml_dtypes

        bf = ml_dtypes.bfloat16
        wo = np.ascontiguousarray(w_out[hbase : hbase + HL * HD, :]).astype(bf)

        in_maps.append(
            {
                "xt": np.ascontiguousarray(x[b, :t].T).astype(bf),
                "wqk": wqk.astype(bf),
                "bqk": bqk_col,
                "wv": wv.astype(bf),
                "wo": wo,
            }
        )
    return in_maps


def unshard(parts, b_out):
    """parts[core] is [NPAIR*t, D] of per-pair partials; sum pairs, then
    the two cores of each batch, then add the output bias."""
    out = np.empty((B, T, D), dtype=np.float32)
    for b in range(B):
        lo = parts[2 * b].reshape(NPAIR, T, D).sum(axis=0)
        hi = parts[2 * b + 1].reshape(NPAIR, T, D).sum(axis=0)
        out[b] = lo + hi
    out += b_out[None, None, :]
    return out


def kernel(x, w_qkv, b_qkv, w_out, b_out):
    x = np.asarray(x, dtype=np.float32)
    w_qkv = np.asarray(w_qkv, dtype=np.float32)
    b_qkv = np.asarray(b_qkv, dtype=np.float32)
    w_out = np.asarray(w_out, dtype=np.float32)
    b_out = np.asarray(b_out, dtype=np.float32)

    nc = build_nc()
    _split_multi_waits(nc)
    in_maps = shard_inputs(x, w_qkv, b_qkv, w_out, b_out)
    res = run_bass_kernel_spmd(nc, in_maps, list(range(NCORES)))
    parts = [np.asarray(res.results[i]["out"]).astype(np.float32) for i in range(NCORES)]
    return unshard(parts, b_out).astype(np.float32)
